# revision 44
# baseline (speedup 1.0000x reference)
"""Trainium2 Bass kernel for ComplexMultiheadAttention (v2, bf16 datapath).

Sharding: core c = b*4 + g  (b = batch 0..1, g = head-group 0..3, 4 heads each).
All complex arithmetic is folded into stacked real matmuls via host-side weight
packing; matmuls run in bf16 (fp32 PSUM accumulate), halving DMA/SBUF traffic
vs fp32r at the same PE cycle count.

Device-side simplifications vs reference math:
  - k-bias dropped: scores_lm = (q_l+bq).(k_m+bk) differs from (q_l+bq).k_m by
    an m-independent row constant, which softmax cancels exactly.
  - v-bias folded into the out-proj bias on host: softmax rows sum to 1, so
    P(v+bv) = Pv + bv, and y = (att+bv)Wo^T + bo = att Wo^T + (bv Wo^T + bo).

Layouts (per core):
  xq/xk    : [128, 16, 2048] bf16 packed stacked-transposed activations
             row e' = k*128+p of [x_r[b].T ; x_i[b].T]
  xv       : [128, 16, 2048] bf16, lc-major repack: xv[p, lc, k*128+c] =
             stk[k*128+p, lc*128+c] so V-phase tiles are contiguous DMAs
  wq/wk    : [128, 16, 512] bf16 packed (complex-stacked projection weight).T
  wv       : [128, 16, 512] bf16 packed stacked V weight (natural out layout)
  wo       : [128, 16, 512] bf16 packed out-proj weight slice for this core
  qs/ks    : SBUF [128, 4, 2048] bf16; per head h rows 0:64 q_r.T, 64:128 q_i.T
  vs       : SBUF [128, 16, 512] bf16  [l-chunk, j], j = h*128 + (r/i)*64 + d
  scores   : S.T layout [key m (partitions), query l (free)], PSUM fp32,
             one [128,1024] 2-bank tile per (h, half, mc) -> single wide exp
  softmax  : exp without max subtraction (scores O(+-20), safe in fp32); row
             sums via ones-matmul over partitions; deferred normalization
  osT      : per-head [128, 2048] bf16 -> AllGather over the 4 cores of the
             batch -> ag_out [2048, 2048] bf16; og tiles prefetched to SBUF
             during attention so the final out-proj never waits on DMA
  y        : [512, 2048] fp32 slice of [y_r.T ; y_i.T] (rows g*512..g*512+512)
"""

import os
import sys

for _p in ("/opt/trn_rl_repo",):
    if os.path.isdir(_p) and _p not in sys.path:
        sys.path.insert(0, _p)

import numpy as np

import concourse.bacc as bacc
import concourse.mybir as mybir
import concourse.tile as tile
from concourse import bass_utils

B, L, E, H = 2, 2048, 1024, 16
D = E // H          # 64
NCORES = 8
GROUPS = 4          # head-groups (tensor parallel inside a batch)
HL = H // GROUPS    # heads per core = 4
EL = HL * 2 * D     # stacked rows per core = 512
KC = 16             # 2048 / 128 contraction chunks
NT = L // 512       # 4 moving tiles over L
MT = EL // 128      # 4 output row tiles

F32 = mybir.dt.float32
BF16 = mybir.dt.bfloat16
EXP = mybir.ActivationFunctionType.Exp
IDENT = mybir.ActivationFunctionType.Identity
MULT = mybir.AluOpType.mult


def build_nc(repeat: int = 1, ag_local: bool = False, loop: int = 0):
    nc = bacc.Bacc("TRN2", target_bir_lowering=False, debug=False,
                   num_devices=NCORES)

    xq = nc.dram_tensor("xq", [128, KC, L], BF16, kind="ExternalInput").ap()
    xk = nc.dram_tensor("xk", [128, KC, L], BF16, kind="ExternalInput").ap()
    xv = nc.dram_tensor("xv", [128, KC, L], BF16, kind="ExternalInput").ap()
    wq = nc.dram_tensor("wq", [128, KC, EL], BF16, kind="ExternalInput").ap()
    wk = nc.dram_tensor("wk", [128, KC, EL], BF16, kind="ExternalInput").ap()
    wv = nc.dram_tensor("wv", [128, KC, EL], BF16, kind="ExternalInput").ap()
    wo = nc.dram_tensor("wo", [128, KC, EL], BF16, kind="ExternalInput").ap()
    ones = nc.dram_tensor("ones", [128, 128], BF16, kind="ExternalInput").ap()
    bq = nc.dram_tensor("bq", [128, MT], F32, kind="ExternalInput").ap()
    bo = nc.dram_tensor("bo", [128, MT], F32, kind="ExternalInput").ap()
    y = nc.dram_tensor("y", [EL, L], F32, kind="ExternalOutput").ap()

    rg = [[0, 1, 2, 3], [4, 5, 6, 7]]

    with tile.TileContext(nc) as tc:
        with tc.tile_pool(name="persist", bufs=1) as persist:
            # persist loads ride the gpsimd DGE queue so they don't delay the
            # critical first weight/x chunks on the sync/scalar queues
            ones_t = persist.tile([128, 128], BF16)
            nc.gpsimd.dma_start(ones_t[:], ones[:])
            bq_t = persist.tile([128, MT], F32)
            nc.gpsimd.dma_start(bq_t[:], bq[:])
            bo_t = persist.tile([128, MT], F32)
            nc.gpsimd.dma_start(bo_t[:], bo[:])

            if loop:
                with tc.For_i(0, loop, 1):
                    _emit_body(nc, tc, 0, xq, xk, xv, wq, wk, wv, wo, y,
                               ones_t, bq_t, bo_t, rg, ag_local=ag_local)
            else:
                for rep in range(repeat):
                    _emit_body(nc, tc, rep, xq, xk, xv, wq, wk, wv, wo, y,
                               ones_t, bq_t, bo_t, rg, ag_local=ag_local)

    nc.compile()
    return nc


def _emit_body(nc, tc, rep, xq, xk, xv, wq, wk, wv, wo, y,
               ones_t, bq_t, bo_t, rg, ag_local=False):
    ag_in = nc.dram_tensor(f"ag_in_{rep}", [EL, L], BF16).ap()
    ag_out = nc.dram_tensor(f"ag_out_{rep}", [GROUPS * EL, L], BF16).ap()
    ag_in_v = ag_in.rearrange("(h p) l -> h p l", p=128)

    from contextlib import ExitStack
    with tc.tile_pool(name="qkv_sb", bufs=1) as qkv_sb, ExitStack() as wstk:
        qs_sb = qkv_sb.tile([128, HL, L], BF16)
        ks_sb = qkv_sb.tile([128, HL, L], BF16)
        vs_sb = qkv_sb.tile([128, KC, EL], BF16)

        # staggered weight prefetch: at most two weight slabs live at once
        wpools = {}
        _wside = {"wq": "left", "wk": "right", "wv": "left", "wo": "right"}

        def w_open(name):
            ctx = tc.tile_pool(name=f"wp_{name}", bufs=1, side=_wside[name])
            pool = ctx.__enter__()
            w_t = [pool.tile([128, 4, EL], BF16, name=f"w_{name}{c}")
                   for c in range(4)]
            wpools[name] = ctx
            return w_t

        def w_dma(w_t, w_d, chunks=range(4), eng=None):
            # the startup-critical chunk rides the scalar queue so it lands
            # in parallel with the first x tile on the sync queue
            for c in chunks:
                (eng or nc.sync).dma_start(w_t[c][:],
                                           w_d[:, c * 4:(c + 1) * 4, :])

        def w_free(name):
            wpools.pop(name).__exit__(None, None, None)

        def w_at(w_t, k):
            return w_t[k // 4][:, k % 4, :]

        # shared x-tile and projection-psum pools across q/k/v phases so the
        # next phase's first DMA prefetches during the current phase
        with tc.tile_pool(name="xp", bufs=4) as xp, \
             tc.tile_pool(name="pp", bufs=6, space="PSUM") as pp, \
             tc.tile_pool(name="vpp", bufs=2, space="PSUM") as vpp:

            def qk_phase(x_d, w_t, out_sb, bias_t, pf, pre=None):
                for n in range(NT):
                    ls = slice(n * 512, (n + 1) * 512)
                    accs = [pp.tile([128, 512], F32, name=f"qk_acc{m}",
                                    tag="qk_acc")
                            for m in range(MT)]
                    for kg in range(KC // 4):
                        xt = xp.tile([128, 4, 512], BF16, name="xqk",
                                     tag="xqk")
                        nc.sync.dma_start(xt[:], x_d[:, kg * 4:(kg + 1) * 4, ls])
                        if n == 0 and kg == 0 and pre is not None:
                            pre()
                        for ki in range(4):
                            k = kg * 4 + ki
                            for m in range(MT):
                                nc.tensor.matmul(
                                    accs[m][:],
                                    w_at(w_t, k)[:, m * 128:(m + 1) * 128],
                                    xt[:, ki, :],
                                    start=(k == 0), stop=(k == KC - 1))
                    for m in range(MT):
                        if bias_t is not None:
                            nc.scalar.activation(out_sb[:, m, ls], accs[m][:],
                                                 IDENT, bias=bias_t[:, m:m + 1])
                        else:
                            nc.scalar.activation(out_sb[:, m, ls], accs[m][:],
                                                 IDENT)
                    if n == 0 and pf is not None:
                        w_dma(*pf)  # prefetch next weight after phase ramp

            # ---------------- Q / K projections ----------------
            wq_t = w_open("wq")
            w_dma(wq_t, wq, chunks=(0,))
            wk_t = w_open("wk")
            qk_phase(xq, wq_t, qs_sb, bq_t, (wk_t, wk),
                     pre=lambda: w_dma(wq_t, wq, chunks=(1, 2, 3)))
            w_free("wq")
            wv_t = w_open("wv")
            qk_phase(xk, wk_t, ks_sb, None, (wv_t, wv))
            w_free("wk")
            wo_t = w_open("wo")

            # ---------------- V projection ----------------
            for lc in range(KC):
                acc = vpp.tile([128, EL], F32, name="v_acc", tag="v_acc")
                xt = xp.tile([128, KC * 128], BF16, name="xv_t", tag="xqk")
                nc.sync.dma_start(xt[:], xv[:, lc, :])
                xt_v = xt.rearrange("p (k c) -> p k c", c=128)
                for k in range(KC):
                    nc.tensor.matmul(acc[:], xt_v[:, k, :], w_at(wv_t, k),
                                     start=(k == 0), stop=(k == KC - 1))
                nc.scalar.activation(vs_sb[:, lc, :], acc[:], IDENT)
                if lc == 0:
                    w_dma(wo_t, wo)
            w_free("wv")

        # ---------------- attention + interleaved out-proj ----------------
        # out-proj for head h-1 is emitted inside head h's window (one-head
        # lag), accumulating into y_acc via DVE so the PE fills the slack of
        # the ACT-bound softmax loop instead of idling in a final phase
        og_t = {}
        with tc.tile_pool(name="ogp", bufs=1) as ogp:
            with tc.tile_pool(name="scp", bufs=2, space="PSUM") as scp, \
                 tc.tile_pool(name="pvp", bufs=2, space="PSUM") as pvp, \
                 tc.tile_pool(name="oap", bufs=2, space="PSUM") as oap, \
                 tc.tile_pool(name="ep", bufs=5) as ep, \
                 tc.tile_pool(name="tsp", bufs=7) as tsp, \
                 tc.tile_pool(name="otp", bufs=3) as otp:

                for h in range(HL):
                    for half in range(2):
                        ns = (2 * half, 2 * half + 1)
                        pv2 = [pvp.tile([128, 512], F32, name=f"pv{j}",
                                        tag="pv")
                               for j in range(2)]
                        # software-pipelined: scores+exp for mc+1 are emitted
                        # before PV of mc, so the PE chews on the next score
                        # tile while ACT computes the current exp
                        def emit_sc(mc):
                            ms = slice(mc * 128, (mc + 1) * 128)
                            sc = scp.tile([128, 1024], F32, name="sc",
                                          tag="sc")
                            for j, n in enumerate(ns):
                                ls = slice(n * 512, (n + 1) * 512)
                                nc.tensor.matmul(sc[:, j * 512:(j + 1) * 512],
                                                 ks_sb[:, h, ms],
                                                 qs_sb[:, h, ls],
                                                 start=True, stop=True)
                            ex = ep.tile([128, 1024], BF16, name="ex")
                            nc.scalar.activation(ex[:], sc[:], EXP,
                                                 scale=float(1.0 / np.sqrt(D)))
                            return ex

                        def emit_pv(mc, ex):
                            for j in range(2):
                                js = slice(j * 512, (j + 1) * 512)
                                nc.tensor.matmul(
                                    pv2[j][:],
                                    vs_sb[:, mc, h * 128:(h + 1) * 128],
                                    ex[:, js],
                                    start=(mc == 0), stop=(mc == KC - 1))

                        # bf16 pairwise adder tree on the (otherwise idle)
                        # DVE replaces 2*KC rowsum matmuls per half with 2:
                        # partition-sum runs once on the tree root
                        levels = [None] * 5

                        def tree_push(t):
                            lvl = 0
                            while levels[lvl] is not None:
                                nt = tsp.tile([128, 1024], BF16, name="ts")
                                nc.vector.tensor_add(nt[:], levels[lvl][:],
                                                     t[:])
                                levels[lvl] = None
                                t = nt
                                lvl += 1
                            levels[lvl] = t

                        ex_prev = emit_sc(0)
                        for mc in range(1, KC):
                            ex = emit_sc(mc)
                            emit_pv(mc - 1, ex_prev)
                            tree_push(ex_prev)
                            ex_prev = ex
                        emit_pv(KC - 1, ex_prev)
                        tree_push(ex_prev)
                        treesum = levels[4]
                        # normalize: ot = pv / colsum -> DRAM ag_in (bf16)
                        for j, n in enumerate(ns):
                            ls = slice(n * 512, (n + 1) * 512)
                            js = slice(j * 512, (j + 1) * 512)
                            rs = oap.tile([128, 512], F32, name="rs",
                                          tag="oacc")
                            nc.tensor.matmul(rs[:], ones_t[:], treesum[:, js],
                                             start=True, stop=True)
                            rbc = ep.tile([128, 512], F32, name="rbc")
                            nc.vector.reciprocal(rbc[:], rs[:])
                            ot = otp.tile([128, 512], BF16, name="ot")
                            nc.vector.tensor_tensor(ot[:], pv2[j][:], rbc[:],
                                                    MULT)
                            nc.sync.dma_start(ag_in_v[h][:, ls], ot[:])
                    # per-head AllGather fires once this head's rows land
                    if ag_local:
                        for g in range(GROUPS):
                            nc.sync.dma_start(
                                ag_out[(h * GROUPS + g) * 128:
                                       (h * GROUPS + g + 1) * 128, :],
                                ag_in_v[h])
                    else:
                        nc.gpsimd.collective_compute(
                            "AllGather", mybir.AluOpType.bypass,
                            replica_groups=rg,
                            ins=[ag_in_v[h].opt()],
                            outs=[ag_out[h * 512:(h + 1) * 512, :].opt()])
                    # prefetch this head's gathered rows for the out-proj
                    for kk in range(4):
                        k = h * 4 + kk
                        og = ogp.tile([128, L], BF16, name=f"og{k}",
                                      tag=f"og{k}")
                        nc.sync.dma_start(og[:],
                                          ag_out[k * 128:(k + 1) * 128, :])
                        og_t[k] = og

            # ---------------- out projection ----------------
            with tc.tile_pool(name="opp", bufs=6, space="PSUM") as opp, \
                 tc.tile_pool(name="yp", bufs=3) as yp:
                for n in range(NT):
                    ls = slice(n * 512, (n + 1) * 512)
                    accs = [opp.tile([128, 512], F32, name=f"o_acc{m}",
                                     tag="o_acc")
                            for m in range(MT)]
                    for k in range(KC):
                        for m in range(MT):
                            nc.tensor.matmul(
                                accs[m][:],
                                w_at(wo_t, k)[:, m * 128:(m + 1) * 128],
                                og_t[k][:, ls],
                                start=(k == 0), stop=(k == KC - 1))
                    for m in range(MT):
                        yt = yp.tile([128, 512], F32, name="yt")
                        nc.scalar.activation(yt[:], accs[m][:], IDENT,
                                             bias=bo_t[:, m:m + 1])
                        nc.sync.dma_start(y[m * 128:(m + 1) * 128, ls], yt[:])
        w_free("wo")


def _pack(a, rows=128):
    """[rows*KC', F] -> [rows, KC', F] with row k*rows+p -> [p, k]."""
    kc = a.shape[0] // rows
    return np.ascontiguousarray(
        a.reshape(kc, rows, *a.shape[1:]).transpose(1, 0, 2))


def _stack_qk_w(Wr, Wi, g):
    """Transposed stacked projection weight [2048, 512] for head-group g."""
    hsl = slice(g * HL * D, (g + 1) * HL * D)
    top = np.concatenate([Wr[hsl].T, -Wi[hsl].T], axis=0)  # part=0 cols
    bot = np.concatenate([Wi[hsl].T, Wr[hsl].T], axis=0)   # part=1 cols
    return np.ascontiguousarray(
        np.stack([top.reshape(2 * E, HL, D), bot.reshape(2 * E, HL, D)],
                 axis=2).reshape(2 * E, EL))


def _stack_bias(br, bi, g):
    hsl = slice(g * HL * D, (g + 1) * HL * D)
    s = np.stack([br[hsl].reshape(HL, D), bi[hsl].reshape(HL, D)],
                 axis=1).reshape(EL)
    return np.ascontiguousarray(s.reshape(MT, 128).T)  # [128, MT]


def _bf16(a):
    import ml_dtypes
    return np.ascontiguousarray(a).astype(ml_dtypes.bfloat16)


def prep_in_maps(inputs):
    f32 = np.float32
    xs = {}
    for b in range(B):
        for nm, xr, xi in (("xq", inputs["query_r"], inputs["query_i"]),
                           ("xk", inputs["key_r"], inputs["key_i"]),
                           ("xv", inputs["value_r"], inputs["value_i"])):
            stk = np.concatenate([np.asarray(xr[b]).T, np.asarray(xi[b]).T],
                                 axis=0).astype(f32)     # [2048, L]
            p = _pack(stk)                               # [128, k, L]
            if nm == "xv":
                # lc-major repack: [128, lc, k*128+c]
                p = p.reshape(128, KC, KC, 128).transpose(0, 2, 1, 3) \
                     .reshape(128, KC, L)
            xs[(nm, b)] = _bf16(p)

    # out-proj: full stacked weight [e''=2048, out_row=2048]
    WoT_r = np.asarray(inputs["Wo_r"]).T.astype(f32)
    WoT_i = np.asarray(inputs["Wo_i"]).T.astype(f32)
    top = np.concatenate([WoT_r, WoT_i], axis=1)    # part=0 rows
    bot = np.concatenate([-WoT_i, WoT_r], axis=1)   # part=1 rows
    inter = np.stack([top.reshape(H, D, 2 * E), bot.reshape(H, D, 2 * E)],
                     axis=1).reshape(2 * E, 2 * E)  # [(head,part,d), row]
    # per-head AllGather lays ag_out out as (h_local, rank) blocks; block
    # b = h_local*GROUPS + rank holds global head rank*HL + h_local
    perm = [(b % GROUPS) * HL + b // GROUPS for b in range(H)]
    inter = inter.reshape(H, 2 * D, 2 * E)[perm].reshape(2 * E, 2 * E)
    # fold the v-bias through the out projection: y = att Wo^T + (bv Wo^T + bo)
    bv_r = np.asarray(inputs["bv_r"], f32)
    bv_i = np.asarray(inputs["bv_i"], f32)
    bo_r = (np.asarray(inputs["bo_r"], f32)
            + bv_r @ np.asarray(inputs["Wo_r"], f32).T
            - bv_i @ np.asarray(inputs["Wo_i"], f32).T)
    bo_i = (np.asarray(inputs["bo_i"], f32)
            + bv_r @ np.asarray(inputs["Wo_i"], f32).T
            + bv_i @ np.asarray(inputs["Wo_r"], f32).T)
    bo_cat = np.concatenate([bo_r, bo_i]).astype(f32)

    ones = np.ones((128, 128), dtype=f32)
    in_maps = []
    for c in range(NCORES):
        b, g = divmod(c, GROUPS)
        m = {
            "xq": xs[("xq", b)], "xk": xs[("xk", b)], "xv": xs[("xv", b)],
            "wq": _bf16(_pack(_stack_qk_w(np.asarray(inputs["Wq_r"], f32),
                                          np.asarray(inputs["Wq_i"], f32),
                                          g))),
            "wk": _bf16(_pack(_stack_qk_w(np.asarray(inputs["Wk_r"], f32),
                                          np.asarray(inputs["Wk_i"], f32),
                                          g))),
            "wv": _bf16(_pack(_stack_qk_w(np.asarray(inputs["Wv_r"], f32),
                                          np.asarray(inputs["Wv_i"], f32),
                                          g))),
            "wo": _bf16(_pack(np.ascontiguousarray(
                inter[:, g * EL:(g + 1) * EL]))),
            "ones": _bf16(ones),
            "bq": _stack_bias(np.asarray(inputs["bq_r"], f32),
                              np.asarray(inputs["bq_i"], f32), g),
            "bo": np.ascontiguousarray(
                bo_cat[g * EL:(g + 1) * EL].reshape(MT, 128).T),
        }
        in_maps.append(m)
    return in_maps


def assemble(results):
    out = np.empty((2, B, L, E), np.float32)
    for b in range(B):
        ys = np.concatenate([results[b * GROUPS + g]["y"]
                             for g in range(GROUPS)], axis=0)  # [2048, L]
        out[0, b] = ys[:E].T
        out[1, b] = ys[E:].T
    return out


_NC_CACHE = {}


def get_nc(repeat: int = 1):
    if repeat not in _NC_CACHE:
        _NC_CACHE[repeat] = build_nc(repeat)
    return _NC_CACHE[repeat]


def make_runner(nc):
    """Build a reusable jitted SPMD executor for `nc` (compiles once).

    Mirrors concourse.bass2jax.run_bass_via_pjrt's multi-core path, but the
    jitted callable is constructed a single time so repeated invocations do
    not re-trigger the walrus/NEFF compile.
    """
    import jax
    from jax.experimental.shard_map import shard_map
    from jax.sharding import Mesh, PartitionSpec

    from concourse import bass2jax

    bass2jax.install_neuronx_cc_hook()
    assert nc.dbg_addr is None

    partition_name = (nc.partition_id_tensor.name
                      if nc.partition_id_tensor else None)
    in_names, out_names, out_avals, zero_outs = [], [], [], []
    for alloc in nc.m.functions[0].allocations:
        if not isinstance(alloc, mybir.MemoryLocationSet):
            continue
        name = alloc.memorylocations[0].name
        if alloc.kind == "ExternalInput":
            if name != partition_name:
                in_names.append(name)
        elif alloc.kind == "ExternalOutput":
            shape = tuple(alloc.tensor_shape)
            dtype = mybir.dt.np(alloc.dtype)
            out_names.append(name)
            out_avals.append(jax.core.ShapedArray(shape, dtype))
            zero_outs.append(np.zeros(shape, dtype))
    n_params = len(in_names)
    n_outs = len(out_avals)
    all_in_names = list(in_names) + list(out_names)
    if partition_name is not None:
        all_in_names.append(partition_name)

    def _body(*args):
        operands = list(args)
        if partition_name is not None:
            operands.append(bass2jax.partition_id_tensor())
        outs = bass2jax._bass_exec_p.bind(
            *operands,
            out_avals=tuple(out_avals),
            in_names=tuple(all_in_names),
            out_names=tuple(out_names),
            lowering_input_output_aliases=(),
            sim_require_finite=True,
            sim_require_nnan=True,
            nc=nc,
        )
        return tuple(outs)

    devices = jax.devices()[:NCORES]
    mesh = Mesh(np.asarray(devices), ("core",))
    specs_in = (PartitionSpec("core"),) * (n_params + n_outs)
    specs_out = (PartitionSpec("core"),) * n_outs
    donate = tuple(range(n_params, n_params + n_outs))
    sharded = jax.jit(
        shard_map(_body, mesh=mesh, in_specs=specs_in, out_specs=specs_out,
                  check_rep=False),
        donate_argnums=donate, keep_unused=True)

    def run(in_maps, device_inputs=None):
        if device_inputs is None:
            device_inputs = put_inputs(in_maps)
        concat_zeros = [
            np.zeros((NCORES * z.shape[0], *z.shape[1:]), z.dtype)
            for z in zero_outs]
        out_arrs = sharded(*device_inputs, *concat_zeros)
        jax.block_until_ready(out_arrs)
        return [
            {name: np.asarray(out_arrs[i]).reshape(
                NCORES, *out_avals[i].shape)[c]
             for i, name in enumerate(out_names)}
            for c in range(NCORES)]

    def put_inputs(in_maps):
        return [
            np.concatenate([np.asarray(in_maps[c][nm])
                            for c in range(NCORES)], axis=0)
            for nm in in_names]

    def put_device(in_maps):
        from jax.sharding import NamedSharding
        sh = NamedSharding(mesh, PartitionSpec("core"))
        arrs = [jax.device_put(a, sh) for a in put_inputs(in_maps)]
        jax.block_until_ready(arrs)
        return arrs

    run.put_inputs = put_inputs
    run.put_device = put_device
    return run


_RUNNER_CACHE = {}


def get_runner(repeat: int = 1):
    if repeat not in _RUNNER_CACHE:
        _RUNNER_CACHE[repeat] = make_runner(get_nc(repeat))
    return _RUNNER_CACHE[repeat]


def kernel(**inputs) -> np.ndarray:
    runner = get_runner(1)
    in_maps = prep_in_maps(inputs)
    results = runner(in_maps)
    return assemble(results)


if __name__ == "__main__":
    pass
